# revision 1
# baseline (speedup 1.0000x reference)
"""Embedding lookup + small linear projection on 8 Trainium2 NeuronCores.

Computation (full problem):
    rows = user_repost_matrix[input.reshape(-1)]      # [12800, 2000] f32
    out  = rows @ W.T + b                             # [12800, 8]
    out.reshape(64, 200, 8)

Distribution strategy: pure data-parallel over the 12800 tokens (1600 per
core). The embedding table is replicated into every core's DRAM, so no
collectives are needed: per-core HBM gather traffic (1600 rows x 8KB =
12.8MB) is identical to a row-sharded layout with all-to-all, minus the
communication.

Per-core device kernel (Tile framework), per 128-token tile (13 tiles):
  1. gpsimd.indirect_dma_start gathers 128 table rows -> SBUF [128, 2000]
  2. PE transposes 16 chunks of [128, 125] -> PSUM [125, 128]
  3. DVE/ACT copy PSUM -> SBUF
  4. PE matmul accumulates C[128 tok, 8] += RT_k^T @ W2_k into PSUM
     (W2 = W.T packed host-side as [125, 16*8] so chunk k is a slice)
  5. DVE adds bias, DMA result slice to DRAM
"""

import sys

if "/opt/trn_rl_repo" not in sys.path:
    sys.path.insert(0, "/opt/trn_rl_repo")

import numpy as np

import concourse.bass as bass
import concourse.tile as tile
from concourse import bacc, mybir
from concourse.bass_utils import run_bass_kernel_spmd
from concourse.masks import make_identity

NTOKEN = 100000
D = 2000
J = 8
B, L = 64, 200
N_CORES = 8
TOK = B * L                      # 12800
PER_CORE = TOK // N_CORES        # 1600
P = 128
TILES = (PER_CORE + P - 1) // P  # 13 (last tile is half-padded)
PAD = TILES * P                  # 1664
KCH = 16                         # feature chunks
KC = D // KCH                    # 125

_cached = None


def _build():
    """Build + compile the SPMD Bass module once."""
    nc = bacc.Bacc(
        "TRN2", target_bir_lowering=False, debug=False, num_devices=N_CORES
    )
    table = nc.dram_tensor(
        "table", [NTOKEN, D], mybir.dt.float32, kind="ExternalInput"
    ).ap()
    idx = nc.dram_tensor(
        "idx", [P, TILES], mybir.dt.int32, kind="ExternalInput"
    ).ap()
    w2 = nc.dram_tensor(
        "w2", [KC, KCH * J], mybir.dt.float32, kind="ExternalInput"
    ).ap()
    bias = nc.dram_tensor(
        "bias", [P, J], mybir.dt.float32, kind="ExternalInput"
    ).ap()
    out = nc.dram_tensor(
        "out", [PAD, J], mybir.dt.float32, kind="ExternalOutput"
    ).ap()

    with tile.TileContext(nc) as tc:
        with (
            tc.tile_pool(name="const", bufs=1) as cpool,
            tc.tile_pool(name="rows", bufs=3) as rpool,
            tc.tile_pool(name="tpsum", bufs=4, space="PSUM") as tppool,
            tc.tile_pool(name="rt", bufs=4) as rtpool,
            tc.tile_pool(name="cpsum", bufs=2, space="PSUM") as cppool,
            tc.tile_pool(name="o", bufs=2) as opool,
        ):
            idx_sb = cpool.tile([P, TILES], mybir.dt.int32)
            nc.sync.dma_start(idx_sb[:], idx[:])
            w2_sb = cpool.tile([KC, KCH * J], mybir.dt.float32)
            nc.sync.dma_start(w2_sb[:], w2[:])
            bias_sb = cpool.tile([P, J], mybir.dt.float32)
            nc.sync.dma_start(bias_sb[:], bias[:])
            ident = cpool.tile([P, P], mybir.dt.float32)
            make_identity(nc, ident[:])

            for i in range(TILES):
                r = rpool.tile([P, D], mybir.dt.float32)
                nc.gpsimd.indirect_dma_start(
                    out=r[:],
                    out_offset=None,
                    in_=table[:],
                    in_offset=bass.IndirectOffsetOnAxis(
                        ap=idx_sb[:, i : i + 1], axis=0
                    ),
                )
                c_ps = cppool.tile([P, J], mybir.dt.float32, space="PSUM")
                for k in range(KCH):
                    t_ps = tppool.tile([KC, P], mybir.dt.float32, space="PSUM")
                    nc.tensor.transpose(
                        out=t_ps[:],
                        in_=r[:, k * KC : (k + 1) * KC],
                        identity=ident[:],
                    )
                    rt = rtpool.tile([KC, P], mybir.dt.float32)
                    if k % 2 == 0:
                        nc.vector.tensor_copy(rt[:], t_ps[:])
                    else:
                        nc.scalar.copy(rt[:], t_ps[:])
                    nc.tensor.matmul(
                        out=c_ps[:],
                        lhsT=rt[:],
                        rhs=w2_sb[:, k * J : (k + 1) * J],
                        start=(k == 0),
                        stop=(k == KCH - 1),
                    )
                o = opool.tile([P, J], mybir.dt.float32)
                nc.vector.tensor_add(o[:], c_ps[:], bias_sb[:])
                nc.sync.dma_start(out[i * P : (i + 1) * P, :], o[:])

    nc.compile()
    return nc


def _get_nc():
    global _cached
    if _cached is None:
        _cached = _build()
    return _cached


def _prep_in_maps(input, user_repost_matrix, W, b):
    idx_full = np.asarray(input).reshape(-1).astype(np.int32)
    table = np.ascontiguousarray(np.asarray(user_repost_matrix, dtype=np.float32))
    Wt = np.asarray(W, dtype=np.float32).T                      # [2000, 8]
    # w2[p, k*8+j] = W.T[k*125+p, j]
    w2 = np.ascontiguousarray(
        Wt.reshape(KCH, KC, J).transpose(1, 0, 2).reshape(KC, KCH * J)
    )
    bias = np.ascontiguousarray(
        np.broadcast_to(np.asarray(b, dtype=np.float32).reshape(1, J), (P, J))
    )
    in_maps = []
    for c in range(N_CORES):
        chunk = idx_full[c * PER_CORE : (c + 1) * PER_CORE]
        padded = np.zeros(PAD, np.int32)
        padded[:PER_CORE] = chunk
        # idx_dram[p, i] = core-local token i*128 + p
        idx_arr = np.ascontiguousarray(padded.reshape(TILES, P).T)
        in_maps.append({"table": table, "idx": idx_arr, "w2": w2, "bias": bias})
    return in_maps


def _run(in_maps, trace=False, **kw):
    nc = _get_nc()
    return run_bass_kernel_spmd(
        nc, in_maps, list(range(N_CORES)), trace=trace, **kw
    )


def _unshard(results):
    parts = [results[c]["out"][:PER_CORE] for c in range(N_CORES)]
    return np.concatenate(parts, axis=0).reshape(B, L, J).astype(np.float32)


def kernel(input, user_repost_matrix, W, b):
    in_maps = _prep_in_maps(input, user_repost_matrix, W, b)
    res = _run(in_maps)
    return _unshard(res.results)


# revision 3
# speedup vs baseline: 1.3633x; 1.3633x over previous
"""Embedding lookup + small linear projection on 8 Trainium2 NeuronCores.

Computation (full problem):
    rows = user_repost_matrix[input.reshape(-1)]      # [12800, 2000] f32
    out  = rows @ W.T + b                             # [12800, 8]
    out.reshape(64, 200, 8)

Distribution strategy: pure data-parallel over the 12800 tokens (1600 per
core). The embedding table is replicated into every core's DRAM, so no
collectives are needed: per-core HBM gather traffic (1600 rows x 8KB =
12.8MB) is identical to a row-sharded layout with all-to-all, minus the
communication.

Per-core device kernel (Tile framework), per 128-token tile (13 tiles):
  1. gpsimd.indirect_dma_start gathers 128 table rows -> SBUF R [128, 2000]
  2. PE transposes 16 chunks of [128, 125] f32 -> PSUM [125, 128] (exact)
  3. Split each transposed chunk into bf16 hi + bf16 residual during the
     PSUM->SBUF copies:  RTh = bf16(psum); RTl = bf16(psum - RTh)
  4. Project with two bf16 matmuls per chunk (3-term compensated product,
     ~1e-5 relative error, ~4x cheaper on PE than fp32):
        c[128,16] += RTh^T @ [W2h | W2l]      (hh and h*lo terms)
        c[128,:8] += RTl^T @ W2h              (lo*h term)
  5. C = c[:, :8] + c[:, 8:] + bias on DVE, DMA result slice to DRAM
"""

import sys

if "/opt/trn_rl_repo" not in sys.path:
    sys.path.insert(0, "/opt/trn_rl_repo")

import ml_dtypes
import numpy as np

import concourse.bass as bass
import concourse.tile as tile
from concourse import bacc, mybir
from concourse.bass_utils import run_bass_kernel_spmd
from concourse.masks import make_identity

NTOKEN = 100000
D = 2000
J = 8
B, L = 64, 200
N_CORES = 8
TOK = B * L                      # 12800
PER_CORE = TOK // N_CORES        # 1600
P = 128
TILES = (PER_CORE + P - 1) // P  # 13 (last tile is half-padded)
PAD = TILES * P                  # 1664
KCH = 16                         # feature chunks
KC = D // KCH                    # 125

F32 = mybir.dt.float32
BF16 = mybir.dt.bfloat16
I32 = mybir.dt.int32

_cached = None


def _build():
    """Build + compile the SPMD Bass module once."""
    nc = bacc.Bacc(
        "TRN2", target_bir_lowering=False, debug=False, num_devices=N_CORES
    )
    table = nc.dram_tensor("table", [NTOKEN, D], F32, kind="ExternalInput").ap()
    idx = nc.dram_tensor("idx", [P, TILES], I32, kind="ExternalInput").ap()
    # w2hl[p, k*16 + j]     = bf16(W.T)[k*125 + p, j]          (hi part)
    # w2hl[p, k*16 + 8 + j] = bf16(W.T - hi)[k*125 + p, j]     (lo part)
    w2hl = nc.dram_tensor("w2hl", [KC, KCH * 2 * J], BF16, kind="ExternalInput").ap()
    bias = nc.dram_tensor("bias", [P, J], F32, kind="ExternalInput").ap()
    out = nc.dram_tensor("out", [PAD, J], F32, kind="ExternalOutput").ap()

    with tile.TileContext(nc) as tc:
        with (
            tc.tile_pool(name="const", bufs=1) as cpool,
            tc.tile_pool(name="rows", bufs=4) as rpool,
            tc.tile_pool(name="tpsum", bufs=6, space="PSUM") as tppool,
            tc.tile_pool(name="rth", bufs=6) as rthpool,
            tc.tile_pool(name="rtl", bufs=6) as rtlpool,
            tc.tile_pool(name="cpsum", bufs=2, space="PSUM") as cppool,
            tc.tile_pool(name="o", bufs=2) as opool,
        ):
            idx_sb = cpool.tile([P, TILES], I32)
            nc.sync.dma_start(idx_sb[:], idx[:])
            w2_sb = cpool.tile([KC, KCH * 2 * J], BF16)
            nc.sync.dma_start(w2_sb[:], w2hl[:])
            bias_sb = cpool.tile([P, J], F32)
            nc.sync.dma_start(bias_sb[:], bias[:])
            ident = cpool.tile([P, P], F32)
            make_identity(nc, ident[:])

            for i in range(TILES):
                r = rpool.tile([P, D], F32)
                nc.gpsimd.indirect_dma_start(
                    out=r[:],
                    out_offset=None,
                    in_=table[:],
                    in_offset=bass.IndirectOffsetOnAxis(
                        ap=idx_sb[:, i : i + 1], axis=0
                    ),
                )
                c_ps = cppool.tile([P, 2 * J], F32, space="PSUM")
                for k in range(KCH):
                    t_ps = tppool.tile([KC, P], F32, space="PSUM")
                    nc.tensor.transpose(
                        out=t_ps[:],
                        in_=r[:, k * KC : (k + 1) * KC],
                        identity=ident[:],
                    )
                    rth = rthpool.tile([KC, P], BF16)
                    # round-to-bf16 copy; alternate DVE/ACT (ACT copy is
                    # ~2x DVE, so give ACT only every other one)
                    if k % 2 == 0:
                        nc.scalar.copy(rth[:], t_ps[:])
                    else:
                        nc.vector.tensor_copy(rth[:], t_ps[:])
                    rtl = rtlpool.tile([KC, P], BF16)
                    nc.vector.tensor_tensor(
                        out=rtl[:],
                        in0=t_ps[:],
                        in1=rth[:],
                        op=mybir.AluOpType.subtract,
                    )
                    nc.tensor.matmul(
                        out=c_ps[:],
                        lhsT=rth[:],
                        rhs=w2_sb[:, k * 2 * J : (k + 1) * 2 * J],
                        start=(k == 0),
                        stop=False,
                        skip_group_check=True,
                    )
                    nc.tensor.matmul(
                        out=c_ps[:, :J],
                        lhsT=rtl[:],
                        rhs=w2_sb[:, k * 2 * J : k * 2 * J + J],
                        start=False,
                        stop=(k == KCH - 1),
                        skip_group_check=True,
                    )
                # combine hh + (hl + lh-term) + bias; one PSUM operand per op
                o = opool.tile([P, J], F32)
                nc.vector.tensor_add(o[:], c_ps[:, :J], bias_sb[:])
                nc.vector.tensor_add(o[:], o[:], c_ps[:, J:])
                nc.sync.dma_start(out[i * P : (i + 1) * P, :], o[:])

    nc.compile()
    return nc


def _get_nc():
    global _cached
    if _cached is None:
        _cached = _build()
    return _cached


def _prep_in_maps(input, user_repost_matrix, W, b):
    idx_full = np.asarray(input).reshape(-1).astype(np.int32)
    table = np.ascontiguousarray(np.asarray(user_repost_matrix, dtype=np.float32))
    Wt = np.asarray(W, dtype=np.float32).T                      # [2000, 8]
    # chunked layout: wc[k][p, j] = W.T[k*125+p, j]
    wc = Wt.reshape(KCH, KC, J)                                  # [16, 125, 8]
    wh = wc.astype(ml_dtypes.bfloat16)
    wl = (wc - wh.astype(np.float32)).astype(ml_dtypes.bfloat16)
    # w2hl[p, k, 0:8] = wh[k, p, :]; w2hl[p, k, 8:16] = wl[k, p, :]
    w2hl = np.concatenate([wh, wl], axis=2)                      # [16, 125, 16]
    w2hl = np.ascontiguousarray(
        w2hl.transpose(1, 0, 2).reshape(KC, KCH * 2 * J)
    )
    bias = np.ascontiguousarray(
        np.broadcast_to(np.asarray(b, dtype=np.float32).reshape(1, J), (P, J))
    )
    in_maps = []
    for c in range(N_CORES):
        chunk = idx_full[c * PER_CORE : (c + 1) * PER_CORE]
        padded = np.zeros(PAD, np.int32)
        padded[:PER_CORE] = chunk
        # idx_dram[p, i] = core-local token i*128 + p
        idx_arr = np.ascontiguousarray(padded.reshape(TILES, P).T)
        in_maps.append(
            {"table": table, "idx": idx_arr, "w2hl": w2hl, "bias": bias}
        )
    return in_maps


def _run(in_maps, trace=False, **kw):
    nc = _get_nc()
    return run_bass_kernel_spmd(
        nc, in_maps, list(range(N_CORES)), trace=trace, **kw
    )


def _unshard(results):
    parts = [results[c]["out"][:PER_CORE] for c in range(N_CORES)]
    return np.concatenate(parts, axis=0).reshape(B, L, J).astype(np.float32)


def kernel(input, user_repost_matrix, W, b):
    in_maps = _prep_in_maps(input, user_repost_matrix, W, b)
    res = _run(in_maps)
    return _unshard(res.results)
